# revision 1
# baseline (speedup 1.0000x reference)
"""CKAFormer Trainium2 kernel: 6 iterations of
    Xn = X / ||X||_row;  P = softmax(relu(Xn@W1+b1)@W2+b2)
    X  = Xn + g*P@(P.T@Xn) - g*Xn@(Xn.T@Xn)
then a final MLP. Row-sharded over 8 NeuronCores; the [dim,dim] and
[out_dim,dim] partial Gram matrices are summed with AllReduce in three bf16
chunks (G cols 0:512, PtX, G cols 512:1024) pipelined against PE work.

Precision scheme: the X state and its row-normalization stay fp32; every
in-loop matmul runs in bf16 (all its error enters the update scaled by
GAMMA=1e-4); the final MLP runs in float32r (tf32-class). GAMMA is folded
into the transposed copies of Xn and W1 is pre-scaled by 1/GAMMA host-side.
"""

import sys

sys.path.insert(0, "/opt/trn_rl_repo")

import ml_dtypes
import numpy as np

import concourse.bass as bass
import concourse.mybir as mybir
import concourse.tile as tile
from concourse.bass_utils import run_bass_kernel_spmd
from concourse.masks import make_identity
from concourse.vector_clock import ScopedClock

DEPTH = 6
GAMMA = 1e-4
DIM = 1024
HIDDEN = 16
OUT_DIM = 64
N = 16384
CORES = 8

NS = N // CORES        # rows per core = 2048
RT = NS // 128         # row tiles = 16
DK = DIM // 128        # dim k-tiles = 8
P = 128

F32 = mybir.dt.float32
F32R = mybir.dt.float32r
BF = mybir.dt.bfloat16
AF = mybir.ActivationFunctionType
ALU = mybir.AluOpType

# this container's walrus only accepts one sync-wait slot per engine
# instruction; hoist excess waits onto preceding EventSemaphore carriers.
_MAX_WAITS = 1


class _TC(tile.TileContext):
    def _drain_and_barrier(self, tick_clock, wait_clock):
        drain_inst = self.nc.sync.drain()
        wait_clock.add_sem_waits(
            drain_inst.ins, ScopedClock({None: tick_clock.global_clock})
        )
        si = drain_inst.ins.sync_info
        w = list(si.on_wait) if si and si.on_wait else []
        if len(w) > 1:
            si.on_wait = w[:1]
            for i in range(1, len(w)):
                c = self.nc.sync.drain()
                c.ins.sync_info = mybir.SyncInfo(on_wait=[w[i]], on_update=[])
        self.nc.all_engine_barrier()
        assert self.sems is not None
        popped = self.nc._tile_sem_poison_stack.pop()
        assert popped is self._sem_poison
        self.nc.clear_and_free_semaphores(list(self.sems.allocated().values()))
        self.nc.all_engine_barrier()


def _split_waits(nc, limit=_MAX_WAITS):
    """Hoist excess sem waits onto EventSemaphore carriers inserted just
    before the over-limit instruction (per-engine program order preserves the
    gating; waits are a conjunction so splitting is sound)."""
    nid = 0
    for bb in nc.main_func.blocks:
        out = []
        changed = False
        for ins in bb.instructions:
            si = ins.sync_info
            w = list(si.on_wait) if si and si.on_wait else []
            if len(w) > limit:
                extra, keep = w[:-limit], w[-limit:]
                for i in range(0, len(extra), limit):
                    ev = mybir.InstEventSemaphore(name=f"wsplit_{nid}", ins=[], outs=[])
                    nid += 1
                    ev.engine = ins.engine
                    ev.sync_info = mybir.SyncInfo(
                        on_wait=extra[i : i + limit], on_update=[]
                    )
                    out.append(ev)
                si.on_wait = keep
                changed = True
            out.append(ins)
        if changed:
            bb.instructions = out


def _build():
    nc = bass.Bass()
    x_ext = nc.declare_dram_parameter("x", [NS, DIM], F32, isOutput=False)
    w1b_ext = nc.declare_dram_parameter("w1b", [DIM, HIDDEN], BF, isOutput=False)
    w2b_ext = nc.declare_dram_parameter("w2b", [HIDDEN, OUT_DIM], BF, isOutput=False)
    w1f_ext = nc.declare_dram_parameter("w1f", [DIM, HIDDEN], F32R, isOutput=False)
    w2f_ext = nc.declare_dram_parameter("w2f", [HIDDEN, OUT_DIM], F32R, isOutput=False)
    b1_ext = nc.declare_dram_parameter("b1", [HIDDEN, 1], F32, isOutput=False)
    b2_ext = nc.declare_dram_parameter("b2", [OUT_DIM, 1], F32, isOutput=False)
    y_ext = nc.declare_dram_parameter("y", [NS, OUT_DIM], F32, isOutput=True)

    with _TC(nc) as tc:
        with (
            tc.tile_pool(name="state", bufs=1) as st,
            tc.tile_pool(name="sq", bufs=1) as sqp,
            tc.tile_pool(name="stg", bufs=4) as stg,
            tc.tile_pool(name="xtmp", bufs=1) as xtp,
            tc.tile_pool(name="ps", bufs=8, space="PSUM") as ps,
            tc.tile_pool(name="dram", bufs=2, space="DRAM") as dram,
        ):
            # persistent state
            xr = [st.tile([P, DIM], F32, name=f"xr{i}", tag=f"xr{i}") for i in range(RT)]
            xb = [st.tile([P, DIM], BF, name=f"xb{i}", tag=f"xb{i}") for i in range(RT)]
            xt = [st.tile([P, NS], BF, name=f"xt{k}", tag=f"xt{k}") for k in range(DK)]
            g = [st.tile([P, DIM], BF, name=f"g{k}", tag=f"g{k}") for k in range(DK)]
            et = st.tile([OUT_DIM, NS], BF, name="et", tag="et")
            er = st.tile([P, RT, OUT_DIM], BF, name="er", tag="er")
            ptx = st.tile([OUT_DIM, DIM], BF, name="ptx", tag="ptx")
            a1 = st.tile([HIDDEN, NS], BF, name="a1", tag="a1")
            a1f = st.tile([HIDDEN, NS], F32, name="a1f", tag="a1f")
            w1b = st.tile([P, DK * HIDDEN], BF, name="w1b", tag="w1b")
            w2b = st.tile([HIDDEN, OUT_DIM], BF, name="w2b", tag="w2b")
            w1f = st.tile([P, DK * HIDDEN], F32R, name="w1f", tag="w1f")
            w2f = st.tile([HIDDEN, OUT_DIM], F32R, name="w2f", tag="w2f")
            b1 = st.tile([HIDDEN, 1], F32, name="b1", tag="b1")
            b2 = st.tile([OUT_DIM, 1], F32, name="b2", tag="b2")
            identb = st.tile([P, P], BF, name="identb", tag="identb")
            identf = st.tile([P, P], F32, name="identf", tag="identf")
            n2 = st.tile([P, RT], F32, name="n2", tag="n2")
            sd = st.tile([P, RT], F32, name="sd", tag="sd")
            inv = st.tile([P, RT], F32, name="inv", tag="inv")
            srow = st.tile([P, RT], F32, name="srow", tag="srow")
            sinv = st.tile([P, RT], F32, name="sinv", tag="sinv")

            # loads
            for i in range(RT):
                nc.sync.dma_start(xr[i][:], x_ext[i * P : (i + 1) * P, :])
            for k in range(DK):
                nc.sync.dma_start(
                    w1b[:, k * HIDDEN : (k + 1) * HIDDEN],
                    w1b_ext[k * P : (k + 1) * P, :],
                )
                nc.sync.dma_start(
                    w1f[:, k * HIDDEN : (k + 1) * HIDDEN],
                    w1f_ext[k * P : (k + 1) * P, :],
                )
            nc.sync.dma_start(w2b[:], w2b_ext[:, :])
            nc.sync.dma_start(w2f[:], w2f_ext[:, :])
            nc.sync.dma_start(b1[:], b1_ext[:, :])
            nc.sync.dma_start(b2[:], b2_ext[:, :])
            make_identity(nc, identb[:])
            make_identity(nc, identf[:])

            def norm_block(i):
                # row norm stats of (raw) block i; xb <- bf16(Xn). xr stays
                # RAW X — the 1/norm scale is folded into the U update.
                sq = sqp.tile([P, DIM], F32, name="sq", tag="sq")
                nc.scalar.activation(
                    sq[:], xr[i][:], AF.Square, accum_out=n2[:, i : i + 1]
                )
                nc.scalar.sqrt(sd[:, i : i + 1], n2[:, i : i + 1])
                nc.vector.reciprocal(inv[:, i : i + 1], sd[:, i : i + 1])
                nc.vector.tensor_scalar_mul(xb[i][:], xr[i][:], inv[:, i : i + 1])

            def phase_gram_half(h, arin):
                # partial (Xn.T @ Xn)[:, h*512:(h+1)*512] in bf16
                for m in range(DK):
                    pg = ps.tile([P, 512], F32, name="ps", tag="ps")
                    for i in range(RT):
                        nc.tensor.matmul(
                            pg[:],
                            xb[i][:, m * P : (m + 1) * P],
                            xb[i][:, h * 512 : (h + 1) * 512],
                            start=(i == 0),
                            stop=(i == RT - 1),
                        )
                    gs = stg.tile([P, 512], BF, name="gs", tag="gs")
                    if h == 0:
                        nc.scalar.copy(gs[:], pg[:])
                    else:
                        nc.vector.tensor_copy(gs[:], pg[:])
                    nc.sync.dma_start(arin[m * P : (m + 1) * P, :], gs[:])

            def phase_transpose():
                # xt[k][:, r] = bf16(Xn[r, k*128:(k+1)*128].T) on PE
                for k in range(DK):
                    for j in range(RT // 4):
                        pt = ps.tile([P, 512], BF, name="ps", tag="ps")
                        for s in range(4):
                            i = 4 * j + s
                            nc.tensor.transpose(
                                pt[:, s * P : (s + 1) * P],
                                xb[i][:, k * P : (k + 1) * P],
                                identb[:],
                            )
                        nc.vector.tensor_scalar_mul(
                            xt[k][:, j * 512 : (j + 1) * 512], pt[:], GAMMA
                        )

            def phase_mlp():
                # a1 = relu(Xn@W1 + b1).T via (W1/g).T @ (g*Xn.T); et = exp(.@W2+b2).T
                for q in range(NS // 512):
                    sl = slice(q * 512, (q + 1) * 512)
                    pa = ps.tile([HIDDEN, 512], F32, name="ps", tag="ps")
                    for k in range(DK):
                        nc.tensor.matmul(
                            pa[:],
                            w1b[:, k * HIDDEN : (k + 1) * HIDDEN],
                            xt[k][:, sl],
                            start=(k == 0),
                            stop=(k == DK - 1),
                        )
                    nc.scalar.activation(a1[:, sl], pa[:], AF.Relu, bias=b1[:])
                    pb = ps.tile([OUT_DIM, 512], F32, name="ps", tag="ps")
                    nc.tensor.matmul(pb[:], w2b[:], a1[:, sl])
                    nc.scalar.activation(et[:, sl], pb[:], AF.Exp, bias=b2[:])

            def phase_et_transpose():
                # er[:, i, :] = et[:, i*128:(i+1)*128].T  (softmax numerators,
                # rows); row sums reduced per half so the softmax denominator
                # chain starts before the second half lands
                pts = []
                for j in range(2):
                    pt = ps.tile([P, 512], BF, name="ps", tag="ps")
                    for s in range(8):
                        i = 8 * j + s
                        nc.tensor.transpose(
                            pt[:, s * OUT_DIM : (s + 1) * OUT_DIM],
                            et[:, i * P : (i + 1) * P],
                            identb[:OUT_DIM, :OUT_DIM],
                        )
                    nc.vector.tensor_reduce(
                        srow[:, 8 * j : 8 * j + 8],
                        pt[:].rearrange("p (i o) -> p i o", o=OUT_DIM),
                        mybir.AxisListType.X,
                        ALU.add,
                    )
                    pts.append(pt)
                return pts

            def phase_p(pts):
                # er[:, i, :] = P rows = sinv * E, fused into the psum->sbuf copy
                nc.vector.reciprocal(sinv[:], srow[:])
                for j in range(2):
                    for s in range(8):
                        i = 8 * j + s
                        nc.vector.tensor_scalar_mul(
                            er[:, i, :],
                            pts[j][:, s * OUT_DIM : (s + 1) * OUT_DIM],
                            sinv[:, i : i + 1],
                        )

            def phase_ptx(arin):
                # partial P.T @ Xn -> [128, 512] staging (rows 64h = col half h)
                for h in range(2):
                    pp = ps.tile([OUT_DIM, 512], F32, name="ps", tag="ps")
                    for i in range(RT):
                        nc.tensor.matmul(
                            pp[:],
                            er[:, i, :],
                            xb[i][:, h * 512 : (h + 1) * 512],
                            start=(i == 0),
                            stop=(i == RT - 1),
                        )
                    pps = stg.tile([OUT_DIM, 512], BF, name="pps", tag="gs")
                    nc.vector.tensor_copy(pps[:], pp[:])
                    nc.sync.dma_start(
                        arin[OUT_DIM * h : OUT_DIM * (h + 1), :], pps[:]
                    )

            def phase_u(h, tail=None):
                # xr[:, cols h] <- Xn[:, cols h] - Xn@(g*G)[:, cols h]
                # (normalization of the raw state fused into the subtract)
                for i in range(RT):
                    xsl = xr[i][:, h * 512 : (h + 1) * 512]
                    pu = ps.tile([P, 512], F32, name="ps", tag="ps")
                    for k in range(DK):
                        nc.tensor.matmul(
                            pu[:],
                            xt[k][:, i * P : (i + 1) * P],
                            g[k][:, h * 512 : (h + 1) * 512],
                            start=(k == 0),
                            stop=(k == DK - 1),
                        )
                    nc.vector.scalar_tensor_tensor(
                        xsl, xsl, inv[:, i : i + 1], pu[:], ALU.mult, ALU.subtract
                    )
                    if tail is not None:
                        tail(i)

            def phase_v(h, tail=None):
                # xr[:, cols h] += sinv * (E @ (g*PtX))[:, cols h]
                # (must run after phase_u(h): the U pass applies the 1/norm)
                for i in range(RT):
                    xsl = xr[i][:, h * 512 : (h + 1) * 512]
                    pv = ps.tile([P, 512], F32, name="ps", tag="ps")
                    nc.tensor.matmul(
                        pv[:],
                        et[:, i * P : (i + 1) * P],
                        ptx[:, h * 512 : (h + 1) * 512],
                    )
                    nc.vector.scalar_tensor_tensor(
                        xsl, pv[:], sinv[:, i : i + 1], xsl, ALU.mult, ALU.add
                    )
                    if tail is not None:
                        tail(i)

            def final_mlp_part(ks):
                # final-MLP A1 partials (f32r): transpose X k-block into xtmp,
                # multiply into a1f accumulated in SBUF via DVE
                for k in ks:
                    xtmp = xtp.tile([P, NS], F32R, name="xtmp", tag="xtmp")
                    for j in range(RT // 4):
                        pt = ps.tile([P, 512], F32, name="ps", tag="ps")
                        for s in range(4):
                            i = 4 * j + s
                            nc.tensor.transpose(
                                pt[:, s * P : (s + 1) * P],
                                xr[i][:, k * P : (k + 1) * P],
                                identf[:],
                            )
                        nc.scalar.mul(xtmp[:, j * 512 : (j + 1) * 512], pt[:], GAMMA)
                    for q in range(4):
                        sl = slice(q * 512, (q + 1) * 512)
                        pa = ps.tile([HIDDEN, 512], F32, name="ps", tag="ps")
                        nc.tensor.matmul(
                            pa[:],
                            w1f[:, k * HIDDEN : (k + 1) * HIDDEN],
                            xtmp[:, sl],
                        )
                        if k == 0:
                            nc.vector.tensor_copy(a1f[:, sl], pa[:])
                        else:
                            nc.vector.tensor_add(a1f[:, sl], a1f[:, sl], pa[:])

            rg = [list(range(CORES))]
            for i in range(RT):
                norm_block(i)

            for it in range(DEPTH):
                arin_a = dram.tile([1024, 512], BF, name="arin_a", tag="arin_a")
                arout_a = dram.tile([1024, 512], BF, name="arout_a", tag="arout_a", addr_space="Shared")
                arin_b = dram.tile([1024, 512], BF, name="arin_b", tag="arin_b")
                arout_b = dram.tile([1024, 512], BF, name="arout_b", tag="arout_b", addr_space="Shared")
                arin_c = dram.tile([P, 512], BF, name="arin_c", tag="arin_c")
                arout_c = dram.tile([P, 512], BF, name="arout_c", tag="arout_c", addr_space="Shared")

                phase_gram_half(0, arin_a)
                nc.gpsimd.collective_compute(
                    "AllReduce", ALU.add,
                    ins=[arin_a.opt()], outs=[arout_a.opt()], replica_groups=rg,
                )
                phase_transpose()
                phase_mlp()
                pts = phase_et_transpose()
                phase_p(pts)
                phase_ptx(arin_c)
                nc.gpsimd.collective_compute(
                    "AllReduce", ALU.add,
                    ins=[arin_c.opt()], outs=[arout_c.opt()], replica_groups=rg,
                )
                phase_gram_half(1, arin_b)
                nc.gpsimd.collective_compute(
                    "AllReduce", ALU.add,
                    ins=[arin_b.opt()], outs=[arout_b.opt()], replica_groups=rg,
                )
                for k in range(DK):
                    nc.sync.dma_start(g[k][:, 0:512], arout_a[k * P : (k + 1) * P, :])
                for h in range(2):
                    nc.sync.dma_start(
                        ptx[:, h * 512 : (h + 1) * 512],
                        arout_c[OUT_DIM * h : OUT_DIM * (h + 1), :],
                    )
                nc.scalar.mul(ptx[:], ptx[:], GAMMA)
                phase_u(0)
                phase_v(0)
                if it == DEPTH - 1:
                    # columns 0:512 of X are final; run half the final-MLP
                    # transposes + A1 partials during the AR-B wait
                    final_mlp_part(range(DK // 2))
                for k in range(DK):
                    nc.sync.dma_start(
                        g[k][:, 512:1024], arout_b[k * P : (k + 1) * P, :]
                    )
                # interleave U and V for cols 512:1024 per block so the ACT
                # norm tail keeps pace with PE (~2.4us/block)
                tail = norm_block if it < DEPTH - 1 else None
                for i in range(RT):
                    xsl = xr[i][:, 512:1024]
                    pu = ps.tile([P, 512], F32, name="ps", tag="ps")
                    for k in range(DK):
                        nc.tensor.matmul(
                            pu[:],
                            xt[k][:, i * P : (i + 1) * P],
                            g[k][:, 512:1024],
                            start=(k == 0),
                            stop=(k == DK - 1),
                        )
                    pv = ps.tile([P, 512], F32, name="ps", tag="ps")
                    nc.tensor.matmul(
                        pv[:], et[:, i * P : (i + 1) * P], ptx[:, 512:1024]
                    )
                    nc.vector.scalar_tensor_tensor(
                        xsl, xsl, inv[:, i : i + 1], pu[:], ALU.mult, ALU.subtract
                    )
                    nc.vector.scalar_tensor_tensor(
                        xsl, pv[:], sinv[:, i : i + 1], xsl, ALU.mult, ALU.add
                    )
                    if tail is not None:
                        tail(i)

            # (part 2: remaining k blocks; part 1 ran inside the last iteration)
            final_mlp_part(range(DK // 2, DK))
            yt = xtp.tile([OUT_DIM, NS], F32, name="yt", tag="yt")
            for q in range(4):
                sl = slice(q * 512, (q + 1) * 512)
                nc.scalar.activation(a1f[:, sl], a1f[:, sl], AF.Relu, bias=b1[:])
                pb = ps.tile([OUT_DIM, 512], F32, name="ps", tag="ps")
                # plain fp32 matmul (4 cyc/row) — only 4 of them, and it
                # sidesteps the fp32r producer-rounding rule for a1f
                nc.tensor.matmul(pb[:], w2f[:].bitcast(F32), a1f[:, sl])
                nc.scalar.activation(yt[:, sl], pb[:], AF.Identity, bias=b2[:])
            # transpose Y.T -> rows and store
            yr = sqp.tile([P, RT, OUT_DIM], F32, name="yr", tag="sq")
            for j in range(2):
                pt = ps.tile([P, 512], F32, name="ps", tag="ps")
                for s in range(8):
                    i = 8 * j + s
                    nc.tensor.transpose(
                        pt[:, s * OUT_DIM : (s + 1) * OUT_DIM],
                        yt[:, i * P : (i + 1) * P].bitcast(F32),
                        identf[:OUT_DIM, :OUT_DIM],
                    )
                nc.vector.tensor_copy(yr[:, 8 * j : 8 * j + 8, :], pt[:])
            nc.sync.dma_start(
                y_ext.rearrange("(i p) o -> p i o", p=P), yr[:, :, :]
            )

    _split_waits(nc)
    return nc


_NC = None


def _get_nc():
    global _NC
    if _NC is None:
        _NC = _build()
    return _NC


def _in_maps(X, W1, b1, W2, b2):
    X = np.asarray(X, dtype=np.float32)
    W1g = np.asarray(W1, dtype=np.float32) / GAMMA
    b1 = np.asarray(b1, dtype=np.float32).reshape(HIDDEN, 1)
    W2 = np.asarray(W2, dtype=np.float32)
    b2 = np.asarray(b2, dtype=np.float32).reshape(OUT_DIM, 1)
    w1b = W1g.astype(ml_dtypes.bfloat16)
    w2b = W2.astype(ml_dtypes.bfloat16)
    return [
        {
            "x": np.ascontiguousarray(X[c * NS : (c + 1) * NS]),
            "w1b": w1b,
            "w2b": w2b,
            "w1f": W1g,
            "w2f": W2,
            "b1": b1,
            "b2": b2,
        }
        for c in range(CORES)
    ]


def run(X, W1, b1, W2, b2, **kwargs):
    nc = _get_nc()
    res = run_bass_kernel_spmd(nc, _in_maps(X, W1, b1, W2, b2), list(range(CORES)), **kwargs)
    out = np.concatenate([res.results[c]["y"] for c in range(CORES)], axis=0)
    return out, res


def kernel(X, W1, b1, W2, b2):
    out, _ = run(X, W1, b1, W2, b2)
    return out

